# revision 1
# baseline (speedup 1.0000x reference)
"""PointWarping v2: fp16 score selection (2x DVE) + exact host re-rank.

Device per core: augmented matmul scores (f32 PSUM) are cast to fp16 on
the PSUM->SBUF copy; DVE max / max_index run at 2x 16-bit throughput and
return the top-8 candidate values+indices per query.  Host re-ranks the
8 candidates with exact f32 distances (reference formula), computes the
weights, gathers neighbor flows and warps.  Queries where the fp16
3rd==8th value ties (candidate set not provably complete) or duplicate
indices appear are recomputed exactly on host (rare).
"""

import numpy as np

B, C, N = 4, 3, 8192
NQ = 4096
NT = 32
EPS = 1e-10
CLAMP = 10.0

_CACHE = {}


def _build():
    if "nc" in _CACHE:
        return _CACHE["nc"]

    from contextlib import ExitStack
    from concourse import bacc, bass, tile
    from concourse import mybir

    nc = bacc.Bacc("TRN2", target_bir_lowering=False, debug=False,
                   enable_asserts=True, num_devices=1)
    f32 = mybir.dt.float32
    f32r = mybir.dt.float32r
    f16 = mybir.dt.float16
    i16 = mybir.dt.int16
    u32 = mybir.dt.uint32
    ADD = mybir.AluOpType.add
    MULT = mybir.AluOpType.mult

    q2 = nc.dram_tensor("q2", [3, NQ], f32, kind="ExternalInput").ap()
    p1 = nc.dram_tensor("p1", [3, N], f32, kind="ExternalInput").ap()
    f1 = nc.dram_tensor("f1", [3, N], f32, kind="ExternalInput").ap()
    p1b = nc.dram_tensor("p1b", [32, 768], f32, kind="ExternalInput").ap()
    f1b = nc.dram_tensor("f1b", [32, 768], f32, kind="ExternalInput").ap()
    vallo = nc.dram_tensor("vallo", [128, 8 * NT], f32,
                           kind="ExternalOutput").ap()
    gidxo = nc.dram_tensor("gidxo", [128, 8 * NT], i16,
                           kind="ExternalOutput").ap()

    with tile.TileContext(nc) as tc, ExitStack() as ctx:
        cp = ctx.enter_context(tc.tile_pool(name="persist", bufs=1))
        spool = ctx.enter_context(tc.tile_pool(name="scores", bufs=2))
        ppool = ctx.enter_context(tc.tile_pool(name="ps", bufs=2, space="PSUM"))
        tp = ctx.enter_context(tc.tile_pool(name="loop", bufs=2))

        def pt(shape, dtype=f32, tag=None):
            return cp.tile(shape, dtype, tag=tag, bufs=1, name=tag or "ptile")

        QSTG = spool.tile([4, NQ], f32, tag="S", name="QSTG")
        nc.vector.memset(QSTG[:, :], -1.0)
        nc.sync.dma_start(QSTG[0:3, :], q2[:, :])
        nc.vector.tensor_scalar(QSTG[0:3, :], QSTG[0:3, :], 2.0, None, MULT)
        QAUG = pt([4, NQ], f32r, tag="QAUG")
        nc.gpsimd.tensor_copy(QAUG[:], QSTG[:])

        KSTG = spool.tile([4, N], f32, tag="S", name="KSTG")
        F1T = pt([3, N], tag="F1T")
        nc.sync.dma_start(KSTG[0:3, :], p1[:, :])
        nc.sync.dma_start(F1T[:], f1[:, :])
        nc.vector.tensor_tensor(KSTG[0:3, :], KSTG[0:3, :], F1T[:], ADD)
        P1B = pt([32, 768], tag="P1B")
        F1B = pt([32, 768], tag="F1B")
        nc.sync.dma_start(P1B[:], p1b[:, :])
        nc.sync.dma_start(F1B[:], f1b[:, :])
        KSQ = pt([32, 768], tag="KSQ")
        nc.vector.tensor_tensor(KSQ[:], P1B[:], F1B[:], ADD)
        nc.scalar.square(KSQ[:], KSQ[:])
        NORM = pt([32, 256], tag="NORM")
        nc.vector.tensor_tensor(NORM[:], KSQ[:, 0:256], KSQ[:, 256:512], ADD)
        nc.vector.tensor_tensor(NORM[:], NORM[:], KSQ[:, 512:768], ADD)
        nc.sync.dma_start(KSTG[3:4, :], NORM[:])
        KAUG = pt([4, N], f32r, tag="KAUG")
        nc.gpsimd.tensor_copy(KAUG[:], KSTG[:])

        VAL8 = pt([128, 8 * NT], tag="VAL8")        # top-8 fp16 scores (as f32)
        GIDX8 = pt([128, 8 * NT], i16, tag="GIDX8")  # top-8 indices

        for t in range(NT):
            S = spool.tile([128, N], f16, tag="S", name="S")
            lhsT = QAUG[:, bass.ts(t, 128)]
            for kc in range(4):
                P = ppool.tile([128, 2048], f32, tag="P", name="P")
                for i in range(4):
                    nc.tensor.matmul(
                        P[:, bass.ts(i, 512)],
                        lhsT,
                        KAUG[:, 2048 * kc + 512 * i:2048 * kc + 512 * (i + 1)],
                        start=True, stop=True)
                nc.scalar.copy(S[:, bass.ts(kc, 2048)], P[:])
            V8 = tp.tile([128, 8], f16, tag="V8", name="V8")
            nc.vector.max(V8[:], S[:])
            I8 = tp.tile([128, 8], u32, tag="I8", name="I8")
            nc.vector.max_index(I8[:], V8[:], S[:])
            nc.gpsimd.tensor_copy(VAL8[:, 8 * t:8 * t + 8], V8[:])
            nc.gpsimd.tensor_copy(GIDX8[:, 8 * t:8 * t + 8], I8[:])

        nc.sync.dma_start(vallo[:, :], VAL8[:])
        nc.sync.dma_start(gidxo[:, :], GIDX8[:])

    nc.compile()
    _CACHE["nc"] = nc
    return nc


def make_core_inputs(pos1, pos2, flow1, core):
    b, h = core // 2, core % 2
    q2 = np.ascontiguousarray(pos2[b, :, h * NQ:(h + 1) * NQ])
    p1 = np.ascontiguousarray(pos1[b])
    f1 = np.ascontiguousarray(flow1[b])
    p1b = np.ascontiguousarray(
        pos1[b].reshape(3, 32, 256).transpose(1, 0, 2).reshape(32, 768))
    f1b = np.ascontiguousarray(
        flow1[b].reshape(3, 32, 256).transpose(1, 0, 2).reshape(32, 768))
    return {"q2": q2, "p1": p1, "f1": f1, "p1b": p1b, "f1b": f1b}


def combine_host(q2, pos1b, flow1b, val8, gidx8):
    """Exact re-rank of device top-8 candidates + weighted warp.

    q2 [3, NQ] queries for this core; pos1b/flow1b [3, 8192];
    val8/gidx8 [128, 8*NT] device outputs (query (t,p) -> row p, cols 8t..).
    Returns [C, NQ] (column q = 128t + p).
    """
    q = np.ascontiguousarray(
        q2.reshape(3, NT, 128).transpose(2, 1, 0)).astype(np.float32)
    idx = np.asarray(gidx8).astype(np.int64).reshape(128, NT, 8)
    v = np.asarray(val8, dtype=np.float32).reshape(128, NT, 8)
    k = (pos1b + flow1b).T.astype(np.float32)            # [8192, 3]
    fl = flow1b.T.astype(np.float32)                     # [8192, 3]

    diff = k[idx] - q[:, :, None, :]                     # [p,t,8,3]
    d2c = (diff * diff).sum(-1, dtype=np.float32)        # [p,t,8]
    order = np.lexsort((idx, d2c), axis=-1)[..., :3]     # by d2 then index
    i3 = np.take_along_axis(idx, order, -1)              # [p,t,3]
    d2_3 = np.take_along_axis(d2c, order, -1)

    # fp16 v3 == v8  =>  candidate set may be incomplete; dup indices too
    vh = v.astype(np.float16)
    flag = vh[..., 2] == vh[..., 7]
    si = np.sort(idx, axis=-1)
    flag |= (np.diff(si, axis=-1) == 0).any(-1)
    if flag.any():
        fp, ft = np.nonzero(flag)
        qf = q[fp, ft]                                   # [m,3]
        d2f = ((qf[:, None, :] - k[None, :, :]) ** 2).sum(-1, dtype=np.float32)
        of = np.argsort(d2f, axis=1, kind="stable")[:, :3]
        i3[fp, ft] = of
        d2_3[fp, ft] = np.take_along_axis(d2f, of, 1)

    dist = np.maximum(np.sqrt(np.maximum(d2_3, 0.0)), EPS).astype(np.float32)
    inv = (1.0 / dist).astype(np.float32)
    w = inv / inv.sum(-1, keepdims=True)
    flow2 = (w[..., None] * fl[i3]).sum(-2, dtype=np.float32)  # [p,t,3]
    res = q - flow2
    np.clip(res, -CLAMP, CLAMP, out=res)
    return res.transpose(2, 1, 0).reshape(C, NQ)


def kernel(pos1, pos2, flow1):
    from concourse.bass_utils import run_bass_kernel_spmd

    pos1 = np.asarray(pos1, dtype=np.float32)
    pos2 = np.asarray(pos2, dtype=np.float32)
    flow1 = np.asarray(flow1, dtype=np.float32)

    nc = _build()
    in_maps = [make_core_inputs(pos1, pos2, flow1, c) for c in range(8)]
    res = run_bass_kernel_spmd(nc, in_maps, core_ids=list(range(8)))

    full = np.empty((B, C, N), dtype=np.float32)
    for core in range(8):
        b, h = core // 2, core % 2
        full[b, :, h * NQ:(h + 1) * NQ] = combine_host(
            in_maps[core]["q2"], pos1[b], flow1[b],
            res.results[core]["vallo"], res.results[core]["gidxo"])
    return full



# revision 2
# speedup vs baseline: 2.0572x; 2.0572x over previous
"""PointWarping v3: fp16 score selection on device + exact host re-rank,
with a cached PJRT executable and fully vectorized host combine.

Device per core: augmented matmul scores (f32 PSUM) are cast to fp16 on
the PSUM->SBUF copy; DVE max / max_index run at 2x 16-bit throughput and
return the top-8 candidate values+indices per query.  Host re-ranks the
8 candidates with exact f32 distances (reference formula), computes the
weights, gathers neighbor flows and warps.  Queries where the fp16
3rd==8th value ties (candidate set not provably complete) or duplicate
indices appear are recomputed exactly on host (rare).

The PJRT shard_map executable is built once and cached; per-call work is
input assembly, one jitted dispatch, and the vectorized combine.
"""

import numpy as np

B, C, N = 4, 3, 8192
NQ = 4096
NT = 32
EPS = 1e-10
CLAMP = 10.0

_CACHE = {}


def _build():
    if "nc" in _CACHE:
        return _CACHE["nc"]

    from contextlib import ExitStack
    from concourse import bacc, bass, tile
    from concourse import mybir

    nc = bacc.Bacc("TRN2", target_bir_lowering=False, debug=False,
                   enable_asserts=True, num_devices=1)
    f32 = mybir.dt.float32
    f32r = mybir.dt.float32r
    f16 = mybir.dt.float16
    i16 = mybir.dt.int16
    u32 = mybir.dt.uint32
    ADD = mybir.AluOpType.add
    MULT = mybir.AluOpType.mult

    q2 = nc.dram_tensor("q2", [3, NQ], f32, kind="ExternalInput").ap()
    p1b = nc.dram_tensor("p1b", [32, 768], f32, kind="ExternalInput").ap()
    f1b = nc.dram_tensor("f1b", [32, 768], f32, kind="ExternalInput").ap()
    vallo = nc.dram_tensor("vallo", [128, 8 * NT], f16,
                           kind="ExternalOutput").ap()
    gidxo = nc.dram_tensor("gidxo", [128, 8 * NT], i16,
                           kind="ExternalOutput").ap()

    with tile.TileContext(nc) as tc, ExitStack() as ctx:
        cp = ctx.enter_context(tc.tile_pool(name="persist", bufs=1))
        spool = ctx.enter_context(tc.tile_pool(name="scores", bufs=2))
        ppool = ctx.enter_context(tc.tile_pool(name="ps", bufs=2, space="PSUM"))
        tp = ctx.enter_context(tc.tile_pool(name="loop", bufs=2))

        def pt(shape, dtype=f32, tag=None):
            return cp.tile(shape, dtype, tag=tag, bufs=1, name=tag or "ptile")

        QSTG = spool.tile([4, NQ], f32, tag="S", name="QSTG")
        nc.vector.memset(QSTG[:, :], -1.0)
        nc.sync.dma_start(QSTG[0:3, :], q2[:, :])
        nc.vector.tensor_scalar(QSTG[0:3, :], QSTG[0:3, :], 2.0, None, MULT)
        QAUG = pt([4, NQ], f32r, tag="QAUG")
        nc.gpsimd.tensor_copy(QAUG[:], QSTG[:])

        P1B = pt([32, 768], tag="P1B")
        F1B = pt([32, 768], tag="F1B")
        nc.sync.dma_start(P1B[:], p1b[:, :])
        nc.sync.dma_start(F1B[:], f1b[:, :])
        KBLK = pt([32, 768], tag="KBLK")
        nc.vector.tensor_tensor(KBLK[:], P1B[:], F1B[:], ADD)

        # [3, N] database layout rebuilt from the blocked form via
        # partition-collapse DMAs (32p x 256 -> 1p x 8192).
        KSTG = spool.tile([4, N], f32, tag="S", name="KSTG")
        for c in range(3):
            nc.sync.dma_start(KSTG[c:c + 1, :], KBLK[:, 256 * c:256 * (c + 1)])

        KSQ = pt([32, 768], tag="KSQ")
        nc.scalar.square(KSQ[:], KBLK[:])
        NORM = pt([32, 256], tag="NORM")
        nc.vector.tensor_tensor(NORM[:], KSQ[:, 0:256], KSQ[:, 256:512], ADD)
        nc.vector.tensor_tensor(NORM[:], NORM[:], KSQ[:, 512:768], ADD)
        nc.sync.dma_start(KSTG[3:4, :], NORM[:])
        KAUG = pt([4, N], f32r, tag="KAUG")
        nc.gpsimd.tensor_copy(KAUG[:], KSTG[:])

        VAL8 = pt([128, 8 * NT], f16, tag="VAL8")    # top-8 fp16 scores
        GIDX8 = pt([128, 8 * NT], i16, tag="GIDX8")  # top-8 indices

        for t in range(NT):
            S = spool.tile([128, N], f16, tag="S", name="S")
            lhsT = QAUG[:, bass.ts(t, 128)]
            for kc in range(4):
                P = ppool.tile([128, 2048], f32, tag="P", name="P")
                for i in range(4):
                    nc.tensor.matmul(
                        P[:, bass.ts(i, 512)],
                        lhsT,
                        KAUG[:, 2048 * kc + 512 * i:2048 * kc + 512 * (i + 1)],
                        start=True, stop=True)
                nc.scalar.copy(S[:, bass.ts(kc, 2048)], P[:])
            V8 = tp.tile([128, 8], f16, tag="V8", name="V8")
            nc.vector.max(V8[:], S[:])
            I8 = tp.tile([128, 8], u32, tag="I8", name="I8")
            nc.vector.max_index(I8[:], V8[:], S[:])
            nc.gpsimd.tensor_copy(VAL8[:, 8 * t:8 * t + 8], V8[:])
            nc.gpsimd.tensor_copy(GIDX8[:, 8 * t:8 * t + 8], I8[:])

        nc.sync.dma_start(vallo[:, :], VAL8[:])
        nc.sync.dma_start(gidxo[:, :], GIDX8[:])

    nc.compile()
    _CACHE["nc"] = nc
    return nc


def _get_runner():
    """Build the 8-core shard_map executable once; return (run, names)."""
    if "runner" in _CACHE:
        return _CACHE["runner"]

    import jax
    import jax.core
    from jax.experimental.shard_map import shard_map
    from jax.sharding import Mesh, PartitionSpec
    from concourse import bass2jax, mybir

    nc = _build()
    bass2jax.install_neuronx_cc_hook()

    dbg_name = None
    if getattr(nc, "dbg_addr", None) is not None:
        if nc.dbg_callbacks:
            raise RuntimeError("dbg_callbacks unsupported under axon")
        dbg_name = nc.dbg_addr.name
    partition_name = (nc.partition_id_tensor.name
                      if nc.partition_id_tensor else None)

    in_names, out_names, out_avals = [], [], []
    for alloc in nc.m.functions[0].allocations:
        if not isinstance(alloc, mybir.MemoryLocationSet):
            continue
        name = alloc.memorylocations[0].name
        if alloc.kind == "ExternalInput":
            if name != partition_name:
                in_names.append(name)
        elif alloc.kind == "ExternalOutput":
            out_names.append(name)
            out_avals.append(jax.core.ShapedArray(
                tuple(alloc.tensor_shape), mybir.dt.np(alloc.dtype)))
    n_params = len(in_names)
    n_outs = len(out_names)
    bind_in_names = list(in_names) + list(out_names)
    if partition_name is not None:
        bind_in_names.append(partition_name)
    donate = tuple(range(n_params, n_params + n_outs))

    def _body(*args):
        operands = list(args)
        if partition_name is not None:
            operands.append(bass2jax.partition_id_tensor())
        outs = bass2jax._bass_exec_p.bind(
            *operands,
            out_avals=tuple(out_avals),
            in_names=tuple(bind_in_names),
            out_names=tuple(out_names),
            lowering_input_output_aliases=(),
            sim_require_finite=True,
            sim_require_nnan=True,
            nc=nc,
        )
        return tuple(outs)

    devices = jax.devices()[:8]
    mesh = Mesh(np.asarray(devices), ("core",))
    in_specs = (PartitionSpec("core"),) * (n_params + n_outs)
    out_specs = (PartitionSpec("core"),) * n_outs
    sharded = jax.jit(
        shard_map(_body, mesh=mesh, in_specs=in_specs,
                  out_specs=out_specs, check_rep=False),
        donate_argnums=donate,
        keep_unused=True,
    )
    zero_shapes = [(tuple(a.shape), a.dtype) for a in out_avals]

    def run(concat_inputs):
        ins = [concat_inputs[n] for n in in_names]
        zeros = [np.zeros((8 * s[0], *s[1:]), d) for s, d in zero_shapes]
        outs = sharded(*ins, *zeros)
        return {name: np.asarray(o) for name, o in zip(out_names, outs)}

    _CACHE["runner"] = (run, dbg_name)
    return _CACHE["runner"]


def _combine_all(pos1, pos2, flow1, val_all, gidx_all):
    """Exact re-rank of device top-8 candidates + weighted warp, all cores.

    val_all: [8*128, 256] f16; gidx_all: [8*128, 256] i16.
    Core c = 2b+h covers pos2[b,:,h*NQ:(h+1)*NQ]; device query (t,p) ->
    row p, cols 8t..8t+7.  Returns full [B, C, N] output.
    """
    v = np.asarray(val_all).reshape(8, 128, NT, 8)            # f16
    idx = np.asarray(gidx_all).astype(np.int32).reshape(8, 128, NT, 8)

    # queries q[core, p, t, c] = pos2[b, c, h*4096 + t*128 + p]
    q = np.ascontiguousarray(
        pos2.reshape(B, C, 2, NT, 128).transpose(0, 2, 4, 3, 1)
    ).reshape(8, 128, NT, C)

    k_flat = np.ascontiguousarray(
        (pos1 + flow1).transpose(0, 2, 1)).reshape(B * N, C)
    fl_flat = np.ascontiguousarray(
        flow1.transpose(0, 2, 1)).reshape(B * N, C)

    boff = (np.arange(8, dtype=np.int32) // 2 * N)[:, None, None, None]
    gidx = idx + boff                                        # [8,128,NT,8]
    diff = k_flat[gidx] - q[:, :, :, None, :]
    d2c = np.einsum('cptkd,cptkd->cptk', diff, diff,
                    dtype=np.float32, casting='unsafe')       # [8,128,NT,8]

    # exact hierarchical sort key: (f32 d2 bits << 13) | idx.  d2 >= 0 so
    # its bit pattern is order-preserving as an unsigned int.
    key = (d2c.view(np.int32).astype(np.int64) << 13) | idx.astype(np.int64)
    key3 = np.sort(key, axis=-1)[..., :3]
    i3 = (key3 & (N - 1)).astype(np.int32)
    d2_3 = (key3 >> 13).astype(np.int32).view(np.float32)

    # fp16 v3 == v8  =>  candidate set may be incomplete; dup indices too
    flag = v[..., 2] == v[..., 7]
    si = np.sort(idx, axis=-1)
    flag |= (np.diff(si, axis=-1) == 0).any(-1)
    if flag.any():
        fc, fp, ft = np.nonzero(flag)
        qf = q[fc, fp, ft]                                   # [m, 3]
        base = (fc // 2) * N
        for j in range(len(fc)):
            kb = k_flat[base[j]:base[j] + N]
            d2f = ((qf[j][None, :] - kb) ** 2).sum(-1, dtype=np.float32)
            kf = (d2f.view(np.int32).astype(np.int64) << 13) \
                | np.arange(N, dtype=np.int64)
            k3 = np.sort(kf)[:3]
            i3[fc[j], fp[j], ft[j]] = (k3 & (N - 1)).astype(np.int32)
            d2_3[fc[j], fp[j], ft[j]] = \
                (k3 >> 13).astype(np.int32).view(np.float32)

    dist = np.maximum(np.sqrt(np.maximum(d2_3, 0.0)), EPS)
    inv = 1.0 / dist
    w = inv / inv.sum(-1, keepdims=True)                     # [8,128,NT,3]
    gfl = fl_flat[i3 + boff]                                 # [8,128,NT,3,3]
    flow2 = np.einsum('cptk,cptkd->cptd', w, gfl)            # [8,128,NT,3]
    res = q - flow2
    np.clip(res, -CLAMP, CLAMP, out=res)
    # res[2b+h, p, t, c] -> full[b, c, h*4096 + t*128 + p]
    return np.ascontiguousarray(
        res.reshape(B, 2, 128, NT, C).transpose(0, 4, 1, 3, 2)
    ).reshape(B, C, N)


def kernel(pos1, pos2, flow1):
    pos1 = np.ascontiguousarray(np.asarray(pos1, dtype=np.float32))
    pos2 = np.ascontiguousarray(np.asarray(pos2, dtype=np.float32))
    flow1 = np.ascontiguousarray(np.asarray(flow1, dtype=np.float32))

    run, dbg_name = _get_runner()

    # per-core [3, NQ] query slabs, concatenated on axis 0
    q2_all = np.ascontiguousarray(
        pos2.reshape(B, C, 2, NQ).transpose(0, 2, 1, 3)).reshape(8 * C, NQ)
    # blocked [32, 768] database/flow layouts, repeated for both halves
    p1b_all = np.repeat(
        pos1.reshape(B, C, 32, 256).transpose(0, 2, 1, 3).reshape(B, 32, 768),
        2, axis=0).reshape(8 * 32, 768)
    f1b_all = np.repeat(
        flow1.reshape(B, C, 32, 256).transpose(0, 2, 1, 3).reshape(B, 32, 768),
        2, axis=0).reshape(8 * 32, 768)

    concat_inputs = {"q2": q2_all, "p1b": p1b_all, "f1b": f1b_all}
    if dbg_name is not None:
        concat_inputs[dbg_name] = np.zeros((8, 2), np.uint32)

    outs = run(concat_inputs)
    return _combine_all(pos1, pos2, flow1, outs["vallo"], outs["gidxo"])


# revision 3
# speedup vs baseline: 3.3878x; 1.6468x over previous
"""PointWarping v3: fp16 score selection on device + exact host re-rank,
with a cached PJRT executable and fully vectorized host combine.

Device per core: augmented matmul scores (f32 PSUM) are cast to fp16 on
the PSUM->SBUF copy; DVE max / max_index run at 2x 16-bit throughput and
return the top-8 candidate values+indices per query.  Host re-ranks the
8 candidates with exact f32 distances (reference formula), computes the
weights, gathers neighbor flows and warps.  Queries where the fp16
3rd==8th value ties (candidate set not provably complete) or duplicate
indices appear are recomputed exactly on host (rare).

The PJRT shard_map executable is built once and cached; per-call work is
input assembly, one jitted dispatch, and the vectorized combine.
"""

import numpy as np

B, C, N = 4, 3, 8192
NQ = 4096
NT = 32
EPS = 1e-10
CLAMP = 10.0

_CACHE = {}


def _build():
    if "nc" in _CACHE:
        return _CACHE["nc"]

    from contextlib import ExitStack
    from concourse import bacc, bass, tile
    from concourse import mybir

    nc = bacc.Bacc("TRN2", target_bir_lowering=False, debug=False,
                   enable_asserts=True, num_devices=1)
    f32 = mybir.dt.float32
    f32r = mybir.dt.float32r
    f16 = mybir.dt.float16
    i16 = mybir.dt.int16
    u32 = mybir.dt.uint32
    ADD = mybir.AluOpType.add
    MULT = mybir.AluOpType.mult

    q2 = nc.dram_tensor("q2", [3, NQ], f32, kind="ExternalInput").ap()
    p1b = nc.dram_tensor("p1b", [32, 768], f32, kind="ExternalInput").ap()
    f1b = nc.dram_tensor("f1b", [32, 768], f32, kind="ExternalInput").ap()
    vallo = nc.dram_tensor("vallo", [128, 8 * NT], f16,
                           kind="ExternalOutput").ap()
    gidxo = nc.dram_tensor("gidxo", [128, 8 * NT], i16,
                           kind="ExternalOutput").ap()

    with tile.TileContext(nc) as tc, ExitStack() as ctx:
        cp = ctx.enter_context(tc.tile_pool(name="persist", bufs=1))
        spool = ctx.enter_context(tc.tile_pool(name="scores", bufs=2))
        ppool = ctx.enter_context(tc.tile_pool(name="ps", bufs=2, space="PSUM"))
        tp = ctx.enter_context(tc.tile_pool(name="loop", bufs=2))

        def pt(shape, dtype=f32, tag=None):
            return cp.tile(shape, dtype, tag=tag, bufs=1, name=tag or "ptile")

        QSTG = spool.tile([4, NQ], f32, tag="S", name="QSTG")
        nc.vector.memset(QSTG[:, :], -1.0)
        nc.sync.dma_start(QSTG[0:3, :], q2[:, :])
        nc.vector.tensor_scalar(QSTG[0:3, :], QSTG[0:3, :], 2.0, None, MULT)
        QAUG = pt([4, NQ], f32r, tag="QAUG")
        nc.gpsimd.tensor_copy(QAUG[:], QSTG[:])

        P1B = pt([32, 768], tag="P1B")
        F1B = pt([32, 768], tag="F1B")
        nc.sync.dma_start(P1B[:], p1b[:, :])
        nc.sync.dma_start(F1B[:], f1b[:, :])
        KBLK = pt([32, 768], tag="KBLK")
        nc.vector.tensor_tensor(KBLK[:], P1B[:], F1B[:], ADD)

        # [3, N] database layout rebuilt from the blocked form via
        # partition-collapse DMAs (32p x 256 -> 1p x 8192).
        KSTG = spool.tile([4, N], f32, tag="S", name="KSTG")
        for c in range(3):
            nc.sync.dma_start(KSTG[c:c + 1, :], KBLK[:, 256 * c:256 * (c + 1)])

        KSQ = pt([32, 768], tag="KSQ")
        nc.scalar.square(KSQ[:], KBLK[:])
        NORM = pt([32, 256], tag="NORM")
        nc.vector.tensor_tensor(NORM[:], KSQ[:, 0:256], KSQ[:, 256:512], ADD)
        nc.vector.tensor_tensor(NORM[:], NORM[:], KSQ[:, 512:768], ADD)
        nc.sync.dma_start(KSTG[3:4, :], NORM[:])
        KAUG = pt([4, N], f32r, tag="KAUG")
        nc.gpsimd.tensor_copy(KAUG[:], KSTG[:])

        VAL8 = pt([128, 8 * NT], f16, tag="VAL8")    # top-8 fp16 scores
        GIDX8 = pt([128, 8 * NT], i16, tag="GIDX8")  # top-8 indices

        for t in range(NT):
            S = spool.tile([128, N], f16, tag="S", name="S")
            lhsT = QAUG[:, bass.ts(t, 128)]
            for kc in range(4):
                P = ppool.tile([128, 2048], f32, tag="P", name="P")
                for i in range(4):
                    nc.tensor.matmul(
                        P[:, bass.ts(i, 512)],
                        lhsT,
                        KAUG[:, 2048 * kc + 512 * i:2048 * kc + 512 * (i + 1)],
                        start=True, stop=True)
                nc.scalar.copy(S[:, bass.ts(kc, 2048)], P[:])
            V8 = tp.tile([128, 8], f16, tag="V8", name="V8")
            nc.vector.max(V8[:], S[:])
            I8 = tp.tile([128, 8], u32, tag="I8", name="I8")
            nc.vector.max_index(I8[:], V8[:], S[:])
            nc.gpsimd.tensor_copy(VAL8[:, 8 * t:8 * t + 8], V8[:])
            nc.gpsimd.tensor_copy(GIDX8[:, 8 * t:8 * t + 8], I8[:])

        nc.sync.dma_start(vallo[:, :], VAL8[:])
        nc.sync.dma_start(gidxo[:, :], GIDX8[:])

    nc.compile()
    _CACHE["nc"] = nc
    return nc


def _get_runner():
    """Build the 8-core shard_map executable once; return (run, names)."""
    if "runner" in _CACHE:
        return _CACHE["runner"]

    import jax
    import jax.core
    from jax.experimental.shard_map import shard_map
    from jax.sharding import Mesh, PartitionSpec
    from concourse import bass2jax, mybir

    nc = _build()
    bass2jax.install_neuronx_cc_hook()

    dbg_name = None
    if getattr(nc, "dbg_addr", None) is not None:
        if nc.dbg_callbacks:
            raise RuntimeError("dbg_callbacks unsupported under axon")
        dbg_name = nc.dbg_addr.name
    partition_name = (nc.partition_id_tensor.name
                      if nc.partition_id_tensor else None)

    in_names, out_names, out_avals = [], [], []
    for alloc in nc.m.functions[0].allocations:
        if not isinstance(alloc, mybir.MemoryLocationSet):
            continue
        name = alloc.memorylocations[0].name
        if alloc.kind == "ExternalInput":
            if name != partition_name:
                in_names.append(name)
        elif alloc.kind == "ExternalOutput":
            out_names.append(name)
            out_avals.append(jax.core.ShapedArray(
                tuple(alloc.tensor_shape), mybir.dt.np(alloc.dtype)))
    n_params = len(in_names)
    n_outs = len(out_names)
    bind_in_names = list(in_names) + list(out_names)
    if partition_name is not None:
        bind_in_names.append(partition_name)
    donate = tuple(range(n_params, n_params + n_outs))

    def _body(*args):
        operands = list(args)
        if partition_name is not None:
            operands.append(bass2jax.partition_id_tensor())
        outs = bass2jax._bass_exec_p.bind(
            *operands,
            out_avals=tuple(out_avals),
            in_names=tuple(bind_in_names),
            out_names=tuple(out_names),
            lowering_input_output_aliases=(),
            sim_require_finite=True,
            sim_require_nnan=True,
            nc=nc,
        )
        return tuple(outs)

    devices = jax.devices()[:8]
    mesh = Mesh(np.asarray(devices), ("core",))
    in_specs = (PartitionSpec("core"),) * (n_params + n_outs)
    out_specs = (PartitionSpec("core"),) * n_outs
    sharded = jax.jit(
        shard_map(_body, mesh=mesh, in_specs=in_specs,
                  out_specs=out_specs, check_rep=False),
        donate_argnums=donate,
        keep_unused=True,
    )
    zero_shapes = [(tuple(a.shape), a.dtype) for a in out_avals]

    def run(concat_inputs):
        ins = [concat_inputs[n] for n in in_names]
        zeros = [np.zeros((8 * s[0], *s[1:]), d) for s, d in zero_shapes]
        outs = sharded(*ins, *zeros)
        # start all D2H copies before the first blocking asarray so the
        # fetches pipeline into a single axon roundtrip
        for o in outs:
            o.copy_to_host_async()
        return {name: np.asarray(o) for name, o in zip(out_names, outs)}

    _CACHE["runner"] = (run, dbg_name)
    return _CACHE["runner"]


def _combine_all(pos1, pos2, flow1, val_all, gidx_all):
    """Exact re-rank of device top-8 candidates + weighted warp, all cores.

    val_all: [8*128, 256] f16; gidx_all: [8*128, 256] i16.
    Core c = 2b+h covers pos2[b,:,h*NQ:(h+1)*NQ]; device query (t,p) ->
    row p, cols 8t..8t+7.  Returns full [B, C, N] output.
    """
    v = np.asarray(val_all).reshape(8, 128, NT, 8)            # f16
    idx = np.asarray(gidx_all).astype(np.int32).reshape(8, 128, NT, 8)

    # queries q[core, p, t, c] = pos2[b, c, h*4096 + t*128 + p]
    q = np.ascontiguousarray(
        pos2.reshape(B, C, 2, NT, 128).transpose(0, 2, 4, 3, 1)
    ).reshape(8, 128, NT, C)

    k_flat = np.ascontiguousarray(
        (pos1 + flow1).transpose(0, 2, 1)).reshape(B * N, C)
    fl_flat = np.ascontiguousarray(
        flow1.transpose(0, 2, 1)).reshape(B * N, C)

    boff = (np.arange(8, dtype=np.int32) // 2 * N)[:, None, None, None]
    gidx = idx + boff                                        # [8,128,NT,8]
    diff = k_flat[gidx] - q[:, :, :, None, :]
    d2c = np.einsum('cptkd,cptkd->cptk', diff, diff,
                    dtype=np.float32, casting='unsafe')       # [8,128,NT,8]

    # exact hierarchical sort key: (f32 d2 bits << 13) | idx.  d2 >= 0 so
    # its bit pattern is order-preserving as an unsigned int.
    key = (d2c.view(np.int32).astype(np.int64) << 13) | idx.astype(np.int64)
    key3 = np.sort(key, axis=-1)[..., :3]
    i3 = (key3 & (N - 1)).astype(np.int32)
    d2_3 = (key3 >> 13).astype(np.int32).view(np.float32)

    # fp16 v3 == v8  =>  candidate set may be incomplete; dup indices too
    flag = v[..., 2] == v[..., 7]
    si = np.sort(idx, axis=-1)
    flag |= (np.diff(si, axis=-1) == 0).any(-1)
    if flag.any():
        fc, fp, ft = np.nonzero(flag)
        qf = q[fc, fp, ft]                                   # [m, 3]
        base = (fc // 2) * N
        for j in range(len(fc)):
            kb = k_flat[base[j]:base[j] + N]
            d2f = ((qf[j][None, :] - kb) ** 2).sum(-1, dtype=np.float32)
            kf = (d2f.view(np.int32).astype(np.int64) << 13) \
                | np.arange(N, dtype=np.int64)
            k3 = np.sort(kf)[:3]
            i3[fc[j], fp[j], ft[j]] = (k3 & (N - 1)).astype(np.int32)
            d2_3[fc[j], fp[j], ft[j]] = \
                (k3 >> 13).astype(np.int32).view(np.float32)

    dist = np.maximum(np.sqrt(np.maximum(d2_3, 0.0)), EPS)
    inv = 1.0 / dist
    w = inv / inv.sum(-1, keepdims=True)                     # [8,128,NT,3]
    gfl = fl_flat[i3 + boff]                                 # [8,128,NT,3,3]
    flow2 = np.einsum('cptk,cptkd->cptd', w, gfl)            # [8,128,NT,3]
    res = q - flow2
    np.clip(res, -CLAMP, CLAMP, out=res)
    # res[2b+h, p, t, c] -> full[b, c, h*4096 + t*128 + p]
    return np.ascontiguousarray(
        res.reshape(B, 2, 128, NT, C).transpose(0, 4, 1, 3, 2)
    ).reshape(B, C, N)


def kernel(pos1, pos2, flow1):
    pos1 = np.ascontiguousarray(np.asarray(pos1, dtype=np.float32))
    pos2 = np.ascontiguousarray(np.asarray(pos2, dtype=np.float32))
    flow1 = np.ascontiguousarray(np.asarray(flow1, dtype=np.float32))

    run, dbg_name = _get_runner()

    # per-core [3, NQ] query slabs, concatenated on axis 0
    q2_all = np.ascontiguousarray(
        pos2.reshape(B, C, 2, NQ).transpose(0, 2, 1, 3)).reshape(8 * C, NQ)
    # blocked [32, 768] database/flow layouts, repeated for both halves
    p1b_all = np.repeat(
        pos1.reshape(B, C, 32, 256).transpose(0, 2, 1, 3).reshape(B, 32, 768),
        2, axis=0).reshape(8 * 32, 768)
    f1b_all = np.repeat(
        flow1.reshape(B, C, 32, 256).transpose(0, 2, 1, 3).reshape(B, 32, 768),
        2, axis=0).reshape(8 * 32, 768)

    concat_inputs = {"q2": q2_all, "p1b": p1b_all, "f1b": f1b_all}
    if dbg_name is not None:
        concat_inputs[dbg_name] = np.zeros((8, 2), np.uint32)

    outs = run(concat_inputs)
    return _combine_all(pos1, pos2, flow1, outs["vallo"], outs["gidxo"])


# revision 6
# speedup vs baseline: 4.0011x; 1.1811x over previous
"""PointWarping v4: fp16 score selection on device + exact host re-rank.

Device per core: augmented matmul scores (f32 PSUM) are cast to fp16 on
the PSUM->SBUF copy; DVE max / max_index run at 2x 16-bit throughput and
return the top-8 candidate values+indices per query.  Host re-ranks the
8 candidates with exact f32 distances (reference formula), computes the
weights, gathers neighbor flows and warps.  Queries where the fp16
3rd==8th value ties (candidate set not provably complete) or duplicate
indices appear are recomputed exactly on host (rare).

Perf notes (axon-tunneled cores: ~80ms RTT, ~170MB/s, ~6ms/exec launch):
- the PJRT shard_map executable is built once and cached
- no donated zero output buffers (kernel writes every output element),
  so nothing but the real inputs is uploaded per call
- all D2H copies start async so both outputs fetch in one roundtrip
- vallo ships only the rank-2 and rank-7 values (the tie flag inputs)
- host combine is vectorized over all 8 cores; 16-byte-row gathers go
  through a complex128 view (single-element fancy indexing)
"""

import numpy as np

B, C, N = 4, 3, 8192
NQ = 4096
NT = 32
EPS = 1e-10
CLAMP = 10.0

_CACHE = {}


def _build():
    if "nc" in _CACHE:
        return _CACHE["nc"]

    from contextlib import ExitStack
    from concourse import bacc, bass, tile
    from concourse import mybir

    nc = bacc.Bacc("TRN2", target_bir_lowering=False, debug=False,
                   enable_asserts=True, num_devices=1)
    f32 = mybir.dt.float32
    f32r = mybir.dt.float32r
    f16 = mybir.dt.float16
    i16 = mybir.dt.int16
    u32 = mybir.dt.uint32
    ADD = mybir.AluOpType.add
    MULT = mybir.AluOpType.mult

    q2 = nc.dram_tensor("q2", [3, NQ], f32, kind="ExternalInput").ap()
    p1b = nc.dram_tensor("p1b", [32, 768], f32, kind="ExternalInput").ap()
    f1b = nc.dram_tensor("f1b", [32, 768], f32, kind="ExternalInput").ap()
    vallo = nc.dram_tensor("vallo", [128, 2 * NT], f16,
                           kind="ExternalOutput").ap()
    gidxo = nc.dram_tensor("gidxo", [128, 8 * NT], i16,
                           kind="ExternalOutput").ap()

    with tile.TileContext(nc) as tc, ExitStack() as ctx:
        cp = ctx.enter_context(tc.tile_pool(name="persist", bufs=1))
        spool = ctx.enter_context(tc.tile_pool(name="scores", bufs=2))
        ppool = ctx.enter_context(tc.tile_pool(name="ps", bufs=2, space="PSUM"))
        tp = ctx.enter_context(tc.tile_pool(name="loop", bufs=2))

        def pt(shape, dtype=f32, tag=None):
            return cp.tile(shape, dtype, tag=tag, bufs=1, name=tag or "ptile")

        QSTG = spool.tile([4, NQ], f32, tag="S", name="QSTG")
        nc.vector.memset(QSTG[:, :], -1.0)
        nc.sync.dma_start(QSTG[0:3, :], q2[:, :])
        nc.vector.tensor_scalar(QSTG[0:3, :], QSTG[0:3, :], 2.0, None, MULT)
        QAUG = pt([4, NQ], f32r, tag="QAUG")
        nc.gpsimd.tensor_copy(QAUG[:], QSTG[:])

        P1B = pt([32, 768], tag="P1B")
        F1B = pt([32, 768], tag="F1B")
        nc.sync.dma_start(P1B[:], p1b[:, :])
        nc.sync.dma_start(F1B[:], f1b[:, :])
        KBLK = pt([32, 768], tag="KBLK")
        nc.vector.tensor_tensor(KBLK[:], P1B[:], F1B[:], ADD)

        # [3, N] database layout rebuilt from the blocked form via
        # partition-collapse DMAs (32p x 256 -> 1p x 8192)
        KSTG = spool.tile([4, N], f32, tag="S", name="KSTG")
        for c in range(3):
            nc.sync.dma_start(KSTG[c:c + 1, :], KBLK[:, 256 * c:256 * (c + 1)])

        KSQ = pt([32, 768], tag="KSQ")
        nc.scalar.square(KSQ[:], KBLK[:])
        NORM = pt([32, 256], tag="NORM")
        nc.vector.tensor_tensor(NORM[:], KSQ[:, 0:256], KSQ[:, 256:512], ADD)
        nc.vector.tensor_tensor(NORM[:], NORM[:], KSQ[:, 512:768], ADD)
        nc.sync.dma_start(KSTG[3:4, :], NORM[:])
        KAUG = pt([4, N], f32r, tag="KAUG")
        nc.gpsimd.tensor_copy(KAUG[:], KSTG[:])

        VAL8 = pt([128, 8 * NT], f16, tag="VAL8")    # top-8 fp16 scores
        GIDX8 = pt([128, 8 * NT], i16, tag="GIDX8")  # top-8 indices

        for t in range(NT):
            S = spool.tile([128, N], f16, tag="S", name="S")
            lhsT = QAUG[:, bass.ts(t, 128)]
            for kc in range(4):
                P = ppool.tile([128, 2048], f32, tag="P", name="P")
                for i in range(4):
                    nc.tensor.matmul(
                        P[:, bass.ts(i, 512)],
                        lhsT,
                        KAUG[:, 2048 * kc + 512 * i:2048 * kc + 512 * (i + 1)],
                        start=True, stop=True)
                nc.scalar.copy(S[:, bass.ts(kc, 2048)], P[:])
            V8 = VAL8[:, 8 * t:8 * t + 8]
            nc.vector.max(V8, S[:])
            I8 = tp.tile([128, 8], u32, tag="I8", name="I8")
            nc.vector.max_index(I8[:], V8, S[:])
            nc.gpsimd.tensor_copy(GIDX8[:, 8 * t:8 * t + 8], I8[:])

        # ship only ranks 2 and 7 of each tile's top-8 (tie-flag inputs)
        V8R = VAL8.rearrange("p (t k) -> p t k", k=8)
        nc.sync.dma_start(vallo[:, 0:NT], V8R[:, :, 2])
        nc.sync.dma_start(vallo[:, NT:2 * NT], V8R[:, :, 7])
        nc.sync.dma_start(gidxo[:, :], GIDX8[:])

    nc.compile()
    _CACHE["nc"] = nc
    return nc


def _get_runner():
    """Build the 8-core shard_map executable once; return (run, dbg_name)."""
    if "runner" in _CACHE:
        return _CACHE["runner"]

    import jax
    import jax.core
    from jax.experimental.shard_map import shard_map
    from jax.sharding import Mesh, PartitionSpec
    from concourse import bass2jax, mybir

    nc = _build()
    bass2jax.install_neuronx_cc_hook()

    dbg_name = None
    if getattr(nc, "dbg_addr", None) is not None:
        if nc.dbg_callbacks:
            raise RuntimeError("dbg_callbacks unsupported under axon")
        dbg_name = nc.dbg_addr.name
    partition_name = (nc.partition_id_tensor.name
                      if nc.partition_id_tensor else None)

    in_names, out_names, out_avals = [], [], []
    for alloc in nc.m.functions[0].allocations:
        if not isinstance(alloc, mybir.MemoryLocationSet):
            continue
        name = alloc.memorylocations[0].name
        if alloc.kind == "ExternalInput":
            if name != partition_name:
                in_names.append(name)
        elif alloc.kind == "ExternalOutput":
            out_names.append(name)
            out_avals.append(jax.core.ShapedArray(
                tuple(alloc.tensor_shape), mybir.dt.np(alloc.dtype)))
    # the kernel writes every element of every output, so no pre-zeroed
    # donated output operands are needed — results are plain custom-call
    # outputs allocated by the runtime
    bind_in_names = list(in_names)
    if partition_name is not None:
        bind_in_names.append(partition_name)

    def _body(*args):
        operands = list(args)
        if partition_name is not None:
            operands.append(bass2jax.partition_id_tensor())
        outs = bass2jax._bass_exec_p.bind(
            *operands,
            out_avals=tuple(out_avals),
            in_names=tuple(bind_in_names),
            out_names=tuple(out_names),
            lowering_input_output_aliases=(),
            sim_require_finite=True,
            sim_require_nnan=True,
            nc=nc,
        )
        return tuple(outs)

    devices = jax.devices()[:8]
    mesh = Mesh(np.asarray(devices), ("core",))
    in_specs = (PartitionSpec("core"),) * len(in_names)
    out_specs = (PartitionSpec("core"),) * len(out_names)
    sharded = jax.jit(
        shard_map(_body, mesh=mesh, in_specs=in_specs,
                  out_specs=out_specs, check_rep=False),
        keep_unused=True,
    )

    def run(concat_inputs):
        outs = sharded(*[concat_inputs[n] for n in in_names])
        # start all D2H copies before the first blocking asarray so the
        # fetches pipeline into a single axon roundtrip
        for o in outs:
            o.copy_to_host_async()
        return {name: np.asarray(o) for name, o in zip(out_names, outs)}

    _CACHE["parts"] = (sharded, list(in_names), list(out_names), mesh)
    _CACHE["runner"] = (run, dbg_name)
    return _CACHE["runner"]


def _combine_all(pos1, pos2, flow1, val_all, gidx_all):
    """Exact re-rank of device top-8 candidates + weighted warp, all cores.

    val_all: [8*128, 64] f16 (cols 0:32 = rank-2 value per tile, 32:64 =
    rank-7); gidx_all: [8*128, 256] i16.  Core c = 2b+h covers
    pos2[b,:,h*NQ:(h+1)*NQ]; device query (t,p) -> row p, cols 8t..8t+7.
    Returns the full [B, C, N] output.
    """
    vv = np.asarray(val_all).reshape(8, 128, 2, NT)
    idx16 = np.asarray(gidx_all).reshape(8, 128, NT, 8)
    idx = idx16.astype(np.int32)

    # queries q[core, p, t, c] = pos2[b, c, h*4096 + t*128 + p]
    q = np.ascontiguousarray(
        pos2.reshape(B, C, 2, NT, 128).transpose(0, 2, 4, 3, 1)
    ).reshape(8, 128, NT, C)

    # 16-byte rows [kx, ky, kz, 0] viewed as complex128 for fast gathers
    kpad = np.zeros((B, N, 4), np.float32)
    kpad[:, :, :3] = (pos1 + flow1).transpose(0, 2, 1)
    kc128 = kpad.reshape(B * N, 4).view(np.complex128).reshape(B * N)
    fpad = np.zeros((B, N, 4), np.float32)
    fpad[:, :, :3] = flow1.transpose(0, 2, 1)
    fc128 = fpad.reshape(B * N, 4).view(np.complex128).reshape(B * N)

    boff = (np.arange(8, dtype=np.int32) // 2 * N)[:, None, None, None]
    gidx = idx + boff                                        # [8,128,NT,8]
    gk = kc128[gidx].view(np.float32).reshape(
        8, 128, NT, 8, 4)[..., :3]
    diff = gk - q[:, :, :, None, :]
    d2c = np.einsum('cptkd,cptkd->cptk', diff, diff,
                    dtype=np.float32, casting='unsafe')      # [8,128,NT,8]

    # exact hierarchical sort key: (f32 d2 bits << 13) | idx.  d2 >= 0 so
    # its bit pattern is order-preserving as an unsigned int.
    key = (d2c.view(np.int32).astype(np.int64) << 13) | idx.astype(np.int64)
    key3 = np.sort(key, axis=-1)[..., :3]
    i3 = (key3 & (N - 1)).astype(np.int32)
    d2_3 = (key3 >> 13).astype(np.int32).view(np.float32)

    # fp16 v3 == v8  =>  candidate set may be incomplete; dup indices too
    flag = vv[:, :, 0, :] == vv[:, :, 1, :]
    si = np.sort(idx16, axis=-1)
    flag |= (np.diff(si, axis=-1) == 0).any(-1)
    if flag.any():
        fc, fp, ft = np.nonzero(flag)
        qf = q[fc, fp, ft]                                   # [m, 3]
        base = (fc // 2) * N
        kall = kpad[..., :3].reshape(B * N, 3)
        for j in range(len(fc)):
            kb = kall[base[j]:base[j] + N]
            d2f = ((qf[j][None, :] - kb) ** 2).sum(-1, dtype=np.float32)
            kf = (d2f.view(np.int32).astype(np.int64) << 13) \
                | np.arange(N, dtype=np.int64)
            k3 = np.sort(kf)[:3]
            i3[fc[j], fp[j], ft[j]] = (k3 & (N - 1)).astype(np.int32)
            d2_3[fc[j], fp[j], ft[j]] = \
                (k3 >> 13).astype(np.int32).view(np.float32)

    dist = np.maximum(np.sqrt(np.maximum(d2_3, 0.0)), EPS)
    inv = 1.0 / dist
    w = inv / inv.sum(-1, keepdims=True)                     # [8,128,NT,3]
    gfl = fc128[i3 + boff].view(np.float32).reshape(
        8, 128, NT, 3, 4)[..., :3]
    flow2 = np.einsum('cptk,cptkd->cptd', w, gfl)            # [8,128,NT,3]
    res = q - flow2
    np.clip(res, -CLAMP, CLAMP, out=res)
    # res[2b+h, p, t, c] -> full[b, c, h*4096 + t*128 + p]
    return np.ascontiguousarray(
        res.reshape(B, 2, 128, NT, C).transpose(0, 4, 1, 3, 2)
    ).reshape(B, C, N)


def kernel(pos1, pos2, flow1):
    pos1 = np.ascontiguousarray(np.asarray(pos1, dtype=np.float32))
    pos2 = np.ascontiguousarray(np.asarray(pos2, dtype=np.float32))
    flow1 = np.ascontiguousarray(np.asarray(flow1, dtype=np.float32))

    run, dbg_name = _get_runner()

    # per-core [3, NQ] query slabs, concatenated on axis 0
    q2_all = np.ascontiguousarray(
        pos2.reshape(B, C, 2, NQ).transpose(0, 2, 1, 3)).reshape(8 * C, NQ)
    # blocked [32, 768] database/flow layouts, repeated for both halves
    p1b_all = np.repeat(
        pos1.reshape(B, C, 32, 256).transpose(0, 2, 1, 3).reshape(B, 32, 768),
        2, axis=0).reshape(8 * 32, 768)
    f1b_all = np.repeat(
        flow1.reshape(B, C, 32, 256).transpose(0, 2, 1, 3).reshape(B, 32, 768),
        2, axis=0).reshape(8 * 32, 768)

    concat_inputs = {"q2": q2_all, "p1b": p1b_all, "f1b": f1b_all}
    if dbg_name is not None:
        concat_inputs[dbg_name] = np.zeros((8, 2), np.uint32)

    outs = run(concat_inputs)
    return _combine_all(pos1, pos2, flow1, outs["vallo"], outs["gidxo"])


# revision 11
# speedup vs baseline: 4.0774x; 1.0191x over previous
"""PointWarping v4: fp16 score selection on device + exact host re-rank.

Device per core: augmented matmul scores (f32 PSUM) are cast to fp16 on
the PSUM->SBUF copy; DVE max / max_index run at 2x 16-bit throughput and
return the top-8 candidate values+indices per query.  Host re-ranks the
8 candidates with exact f32 distances (reference formula), computes the
weights, gathers neighbor flows and warps.  Queries where the fp16
3rd==8th value ties (candidate set not provably complete) or duplicate
indices appear are recomputed exactly on host (rare).

Perf notes (axon-tunneled cores: ~80ms RTT, ~170MB/s, ~6ms/exec launch):
- the PJRT shard_map executable is built once and cached
- no donated zero output buffers (kernel writes every output element),
  so nothing but the real inputs is uploaded per call
- all D2H copies start async so both outputs fetch in one roundtrip
- vallo ships only the rank-2 and rank-7 values (the tie flag inputs)
- host combine is vectorized over all 8 cores; 16-byte-row gathers go
  through a complex128 view (single-element fancy indexing)
"""

import numpy as np

B, C, N = 4, 3, 8192
NQ = 4096
NT = 32
EPS = 1e-10
CLAMP = 10.0

_CACHE = {}


def _build():
    if "nc" in _CACHE:
        return _CACHE["nc"]

    from contextlib import ExitStack
    from concourse import bacc, bass, tile
    from concourse import mybir

    nc = bacc.Bacc("TRN2", target_bir_lowering=False, debug=False,
                   enable_asserts=True, num_devices=1)
    f32 = mybir.dt.float32
    f32r = mybir.dt.float32r
    f16 = mybir.dt.float16
    i16 = mybir.dt.int16
    u32 = mybir.dt.uint32
    ADD = mybir.AluOpType.add
    MULT = mybir.AluOpType.mult

    q2 = nc.dram_tensor("q2", [3, NQ], f16, kind="ExternalInput").ap()
    p1b = nc.dram_tensor("p1b", [32, 768], f16, kind="ExternalInput").ap()
    f1b = nc.dram_tensor("f1b", [32, 768], f16, kind="ExternalInput").ap()
    vallo = nc.dram_tensor("vallo", [128, 2 * NT], f16,
                           kind="ExternalOutput").ap()
    gidxo = nc.dram_tensor("gidxo", [128, 8 * NT], i16,
                           kind="ExternalOutput").ap()

    with tile.TileContext(nc) as tc, ExitStack() as ctx:
        cp = ctx.enter_context(tc.tile_pool(name="persist", bufs=1))
        spool = ctx.enter_context(tc.tile_pool(name="scores", bufs=2))
        ppool = ctx.enter_context(tc.tile_pool(name="ps", bufs=2, space="PSUM"))
        tp = ctx.enter_context(tc.tile_pool(name="loop", bufs=2))

        def pt(shape, dtype=f32, tag=None):
            return cp.tile(shape, dtype, tag=tag, bufs=1, name=tag or "ptile")

        QSTGH = spool.tile([3, NQ], f16, tag="S", name="QSTGH")
        nc.sync.dma_start(QSTGH[:, :], q2[:, :])
        QSTG = spool.tile([4, NQ], f32, tag="S", name="QSTG")
        nc.vector.memset(QSTG[:, :], -1.0)
        nc.vector.tensor_scalar(QSTG[0:3, :], QSTGH[:], 2.0, None, MULT)
        QAUG = pt([4, NQ], f32r, tag="QAUG")
        nc.gpsimd.tensor_copy(QAUG[:], QSTG[:])

        P1B = pt([32, 768], f16, tag="P1B")
        F1B = pt([32, 768], f16, tag="F1B")
        nc.sync.dma_start(P1B[:], p1b[:, :])
        nc.sync.dma_start(F1B[:], f1b[:, :])
        KBLK = pt([32, 768], tag="KBLK")
        nc.vector.tensor_tensor(KBLK[:], P1B[:], F1B[:], ADD)

        # [3, N] database layout rebuilt from the blocked form via
        # partition-collapse DMAs (32p x 256 -> 1p x 8192)
        KSTG = spool.tile([4, N], f32, tag="S", name="KSTG")
        for c in range(3):
            nc.sync.dma_start(KSTG[c:c + 1, :], KBLK[:, 256 * c:256 * (c + 1)])

        KSQ = pt([32, 768], tag="KSQ")
        nc.scalar.square(KSQ[:], KBLK[:])
        NORM = pt([32, 256], tag="NORM")
        nc.vector.tensor_tensor(NORM[:], KSQ[:, 0:256], KSQ[:, 256:512], ADD)
        nc.vector.tensor_tensor(NORM[:], NORM[:], KSQ[:, 512:768], ADD)
        nc.sync.dma_start(KSTG[3:4, :], NORM[:])
        KAUG = pt([4, N], f32r, tag="KAUG")
        nc.gpsimd.tensor_copy(KAUG[:], KSTG[:])

        VAL8 = pt([128, 8 * NT], f16, tag="VAL8")    # top-8 fp16 scores
        GIDX8 = pt([128, 8 * NT], i16, tag="GIDX8")  # top-8 indices

        for t in range(NT):
            S = spool.tile([128, N], f16, tag="S", name="S")
            lhsT = QAUG[:, bass.ts(t, 128)]
            for kc in range(4):
                P = ppool.tile([128, 2048], f32, tag="P", name="P")
                for i in range(4):
                    nc.tensor.matmul(
                        P[:, bass.ts(i, 512)],
                        lhsT,
                        KAUG[:, 2048 * kc + 512 * i:2048 * kc + 512 * (i + 1)],
                        start=True, stop=True)
                nc.scalar.copy(S[:, bass.ts(kc, 2048)], P[:])
            V8 = VAL8[:, 8 * t:8 * t + 8]
            nc.vector.max(V8, S[:])
            I8 = tp.tile([128, 8], u32, tag="I8", name="I8")
            nc.vector.max_index(I8[:], V8, S[:])
            nc.gpsimd.tensor_copy(GIDX8[:, 8 * t:8 * t + 8], I8[:])

        # ship only ranks 2 and 7 of each tile's top-8 (tie-flag inputs)
        V8R = VAL8.rearrange("p (t k) -> p t k", k=8)
        nc.sync.dma_start(vallo[:, 0:NT], V8R[:, :, 2])
        nc.sync.dma_start(vallo[:, NT:2 * NT], V8R[:, :, 7])
        nc.sync.dma_start(gidxo[:, :], GIDX8[:])

    nc.compile()
    _CACHE["nc"] = nc
    return nc


def _get_runner():
    """Build the 8-core shard_map executable once; return (run, dbg_name)."""
    if "runner" in _CACHE:
        return _CACHE["runner"]

    import jax
    import jax.core
    from jax.experimental.shard_map import shard_map
    from jax.sharding import Mesh, PartitionSpec
    from concourse import bass2jax, mybir

    nc = _build()
    bass2jax.install_neuronx_cc_hook()

    dbg_name = None
    if getattr(nc, "dbg_addr", None) is not None:
        if nc.dbg_callbacks:
            raise RuntimeError("dbg_callbacks unsupported under axon")
        dbg_name = nc.dbg_addr.name
    partition_name = (nc.partition_id_tensor.name
                      if nc.partition_id_tensor else None)

    in_names, out_names, out_avals = [], [], []
    for alloc in nc.m.functions[0].allocations:
        if not isinstance(alloc, mybir.MemoryLocationSet):
            continue
        name = alloc.memorylocations[0].name
        if alloc.kind == "ExternalInput":
            if name != partition_name:
                in_names.append(name)
        elif alloc.kind == "ExternalOutput":
            out_names.append(name)
            out_avals.append(jax.core.ShapedArray(
                tuple(alloc.tensor_shape), mybir.dt.np(alloc.dtype)))
    # the kernel writes every element of every output, so no pre-zeroed
    # donated output operands are needed — results are plain custom-call
    # outputs allocated by the runtime
    bind_in_names = list(in_names)
    if partition_name is not None:
        bind_in_names.append(partition_name)

    def _body(*args):
        operands = list(args)
        if partition_name is not None:
            operands.append(bass2jax.partition_id_tensor())
        outs = bass2jax._bass_exec_p.bind(
            *operands,
            out_avals=tuple(out_avals),
            in_names=tuple(bind_in_names),
            out_names=tuple(out_names),
            lowering_input_output_aliases=(),
            sim_require_finite=True,
            sim_require_nnan=True,
            nc=nc,
        )
        return tuple(outs)

    devices = jax.devices()[:8]
    mesh = Mesh(np.asarray(devices), ("core",))
    in_specs = (PartitionSpec("core"),) * len(in_names)
    out_specs = (PartitionSpec("core"),) * len(out_names)
    sharded = jax.jit(
        shard_map(_body, mesh=mesh, in_specs=in_specs,
                  out_specs=out_specs, check_rep=False),
        keep_unused=True,
    )

    def run(concat_inputs):
        outs = sharded(*[concat_inputs[n] for n in in_names])
        # start all D2H copies before the first blocking asarray so the
        # fetches pipeline into a single axon roundtrip
        for o in outs:
            o.copy_to_host_async()
        return {name: np.asarray(o) for name, o in zip(out_names, outs)}

    _CACHE["parts"] = (sharded, list(in_names), list(out_names), mesh)
    _CACHE["runner"] = (run, dbg_name)
    return _CACHE["runner"]


def _combine_all(pos1, pos2, flow1, val_all, gidx_all):
    """Exact re-rank of device top-8 candidates + weighted warp, all cores.

    val_all: [8*128, 64] f16 (cols 0:32 = rank-2 value per tile, 32:64 =
    rank-7); gidx_all: [8*128, 256] i16.  Core c = 2b+h covers
    pos2[b,:,h*NQ:(h+1)*NQ]; device query (t,p) -> row p, cols 8t..8t+7.
    Returns the full [B, C, N] output.
    """
    vv = np.asarray(val_all).reshape(8, 128, 2, NT)
    idx16 = np.asarray(gidx_all).reshape(8, 128, NT, 8)
    idx = idx16.astype(np.int32)

    # queries q[core, p, t, c] = pos2[b, c, h*4096 + t*128 + p]
    q = np.ascontiguousarray(
        pos2.reshape(B, C, 2, NT, 128).transpose(0, 2, 4, 3, 1)
    ).reshape(8, 128, NT, C)

    # 16-byte rows [kx, ky, kz, |k|^2] viewed as complex128 for fast
    # single-element gathers
    kpad = np.empty((B, N, 4), np.float32)
    kpad[:, :, :3] = (pos1 + flow1).transpose(0, 2, 1)
    kpad[:, :, 3] = np.einsum('bnd,bnd->bn', kpad[..., :3], kpad[..., :3])
    kc128 = kpad.reshape(B * N, 4).view(np.complex128).reshape(B * N)
    fpad = np.zeros((B, N, 4), np.float32)
    fpad[:, :, :3] = flow1.transpose(0, 2, 1)
    fc128 = fpad.reshape(B * N, 4).view(np.complex128).reshape(B * N)

    boff = (np.arange(8, dtype=np.int32) // 2 * N)[:, None, None, None]
    gidx = idx + boff                                        # [8,128,NT,8]
    gk = kc128[gidx].view(np.float32).reshape(8, 128, NT, 8, 4)
    # d2 in the reference's dot form: |q|^2 - 2 q.k + |k|^2
    dot = np.einsum('cptkd,cptd->cptk', gk[..., :3], q,
                    dtype=np.float32, casting='unsafe')
    q2s = np.einsum('cptd,cptd->cpt', q, q)
    d2c = gk[..., 3] - 2.0 * dot + q2s[..., None]            # [8,128,NT,8]

    # exact hierarchical sort key: (f32 d2 bits << 13) | idx.  d2 >= 0 so
    # its bit pattern is order-preserving as an unsigned int.
    key = (d2c.view(np.int32).astype(np.int64) << 13) | idx.astype(np.int64)
    key3 = np.sort(key, axis=-1)[..., :3]
    i3 = (key3 & (N - 1)).astype(np.int32)
    d2_3 = (key3 >> 13).astype(np.int32).view(np.float32)

    # fp16 v3 == v8  =>  candidate set may be incomplete; dup indices too.
    # dups only arise from tied fp16 values, which are adjacent in the
    # sorted top-8, so an adjacency check is exact.
    flag = vv[:, :, 0, :] == vv[:, :, 1, :]
    flag |= (idx16[..., 1:] == idx16[..., :-1]).any(-1)
    if flag.any():
        fc, fp, ft = np.nonzero(flag)
        qf = q[fc, fp, ft]                                   # [m, 3]
        base = (fc // 2) * N
        kall = kpad[..., :3].reshape(B * N, 3)
        for j in range(len(fc)):
            kb = kall[base[j]:base[j] + N]
            d2f = ((qf[j][None, :] - kb) ** 2).sum(-1, dtype=np.float32)
            kf = (d2f.view(np.int32).astype(np.int64) << 13) \
                | np.arange(N, dtype=np.int64)
            k3 = np.sort(kf)[:3]
            i3[fc[j], fp[j], ft[j]] = (k3 & (N - 1)).astype(np.int32)
            d2_3[fc[j], fp[j], ft[j]] = \
                (k3 >> 13).astype(np.int32).view(np.float32)

    dist = np.maximum(np.sqrt(np.maximum(d2_3, 0.0)), EPS)
    inv = 1.0 / dist
    w = inv / inv.sum(-1, keepdims=True)                     # [8,128,NT,3]
    gfl = fc128[i3 + boff].view(np.float32).reshape(
        8, 128, NT, 3, 4)[..., :3]
    flow2 = np.einsum('cptk,cptkd->cptd', w, gfl)            # [8,128,NT,3]
    res = q - flow2
    np.clip(res, -CLAMP, CLAMP, out=res)
    # res[2b+h, p, t, c] -> full[b, c, h*4096 + t*128 + p]
    return np.ascontiguousarray(
        res.reshape(B, 2, 128, NT, C).transpose(0, 4, 1, 3, 2)
    ).reshape(B, C, N)


def kernel(pos1, pos2, flow1):
    pos1 = np.ascontiguousarray(np.asarray(pos1, dtype=np.float32))
    pos2 = np.ascontiguousarray(np.asarray(pos2, dtype=np.float32))
    flow1 = np.ascontiguousarray(np.asarray(flow1, dtype=np.float32))

    run, dbg_name = _get_runner()

    # per-core [3, NQ] query slabs, concatenated on axis 0 (f16 upload —
    # selection only; the exact host re-rank uses the f32 originals)
    q2_all = pos2.reshape(B, C, 2, NQ).transpose(0, 2, 1, 3).astype(
        np.float16).reshape(8 * C, NQ)
    # blocked [32, 768] database/flow layouts, repeated for both halves
    p1b_all = np.repeat(
        pos1.reshape(B, C, 32, 256).transpose(0, 2, 1, 3).astype(
            np.float16).reshape(B, 32, 768), 2, axis=0).reshape(8 * 32, 768)
    f1b_all = np.repeat(
        flow1.reshape(B, C, 32, 256).transpose(0, 2, 1, 3).astype(
            np.float16).reshape(B, 32, 768), 2, axis=0).reshape(8 * 32, 768)

    concat_inputs = {"q2": q2_all, "p1b": p1b_all, "f1b": f1b_all}
    if dbg_name is not None:
        concat_inputs[dbg_name] = np.zeros((8, 2), np.uint32)

    outs = run(concat_inputs)
    return _combine_all(pos1, pos2, flow1, outs["vallo"], outs["gidxo"])


# revision 14
# speedup vs baseline: 6.0617x; 1.4867x over previous
"""PointWarping v4: fp16 score selection on device + exact host re-rank.

Device per core: augmented matmul scores (f32 PSUM) are cast to fp16 on
the PSUM->SBUF copy; DVE max / max_index run at 2x 16-bit throughput and
return the top-8 candidate values+indices per query.  Host re-ranks the
8 candidates with exact f32 distances (reference formula), computes the
weights, gathers neighbor flows and warps.  Queries where the fp16
3rd==8th value ties (candidate set not provably complete) or duplicate
indices appear are recomputed exactly on host (rare).

Perf notes (axon-tunneled cores: ~80ms RTT, ~170MB/s, ~6ms/exec launch):
- the PJRT shard_map executable is built once and cached
- no donated zero output buffers (kernel writes every output element),
  so nothing but the real inputs is uploaded per call
- all D2H copies start async so both outputs fetch in one roundtrip
- vallo ships only the rank-2 and rank-7 values (the tie flag inputs)
- host combine is vectorized over all 8 cores; 16-byte-row gathers go
  through a complex128 view (single-element fancy indexing)
"""

import numpy as np

B, C, N = 4, 3, 8192
NQ = 4096
NT = 32
EPS = 1e-10
CLAMP = 10.0

_CACHE = {}


def _build():
    if "nc" in _CACHE:
        return _CACHE["nc"]

    from contextlib import ExitStack
    from concourse import bacc, bass, tile
    from concourse import mybir

    nc = bacc.Bacc("TRN2", target_bir_lowering=False, debug=False,
                   enable_asserts=True, num_devices=1)
    f32 = mybir.dt.float32
    f32r = mybir.dt.float32r
    f16 = mybir.dt.float16
    i16 = mybir.dt.int16
    u32 = mybir.dt.uint32
    ADD = mybir.AluOpType.add
    MULT = mybir.AluOpType.mult

    q2 = nc.dram_tensor("q2", [3, NQ], f16, kind="ExternalInput").ap()
    kb = nc.dram_tensor("kb", [32, 768], f16, kind="ExternalInput").ap()
    vallo = nc.dram_tensor("vallo", [128, 2 * NT], f16,
                           kind="ExternalOutput").ap()
    gidxo = nc.dram_tensor("gidxo", [128, 8 * NT], i16,
                           kind="ExternalOutput").ap()

    with tile.TileContext(nc) as tc, ExitStack() as ctx:
        cp = ctx.enter_context(tc.tile_pool(name="persist", bufs=1))
        spool = ctx.enter_context(tc.tile_pool(name="scores", bufs=2))
        ppool = ctx.enter_context(tc.tile_pool(name="ps", bufs=2, space="PSUM"))
        tp = ctx.enter_context(tc.tile_pool(name="loop", bufs=2))

        def pt(shape, dtype=f32, tag=None):
            return cp.tile(shape, dtype, tag=tag, bufs=1, name=tag or "ptile")

        QSTGH = spool.tile([3, NQ], f16, tag="S", name="QSTGH")
        nc.sync.dma_start(QSTGH[:, :], q2[:, :])
        QSTG = spool.tile([4, NQ], f32, tag="S", name="QSTG")
        nc.vector.memset(QSTG[:, :], -1.0)
        nc.vector.tensor_scalar(QSTG[0:3, :], QSTGH[:], 2.0, None, MULT)
        QAUG = pt([4, NQ], f32r, tag="QAUG")
        nc.gpsimd.tensor_copy(QAUG[:], QSTG[:])

        KBH = pt([32, 768], f16, tag="KBH")
        nc.sync.dma_start(KBH[:], kb[:, :])
        KBLK = pt([32, 768], tag="KBLK")
        nc.scalar.copy(KBLK[:], KBH[:])

        # [3, N] database layout rebuilt from the blocked form via
        # partition-collapse DMAs (32p x 256 -> 1p x 8192)
        KSTG = spool.tile([4, N], f32, tag="S", name="KSTG")
        for c in range(3):
            nc.sync.dma_start(KSTG[c:c + 1, :], KBLK[:, 256 * c:256 * (c + 1)])

        KSQ = pt([32, 768], tag="KSQ")
        nc.scalar.square(KSQ[:], KBLK[:])
        NORM = pt([32, 256], tag="NORM")
        nc.vector.tensor_tensor(NORM[:], KSQ[:, 0:256], KSQ[:, 256:512], ADD)
        nc.vector.tensor_tensor(NORM[:], NORM[:], KSQ[:, 512:768], ADD)
        nc.sync.dma_start(KSTG[3:4, :], NORM[:])
        KAUG = pt([4, N], f32r, tag="KAUG")
        nc.gpsimd.tensor_copy(KAUG[:], KSTG[:])

        VAL8 = pt([128, 8 * NT], f16, tag="VAL8")    # top-8 fp16 scores
        GIDX8 = pt([128, 8 * NT], i16, tag="GIDX8")  # top-8 indices

        for t in range(NT):
            S = spool.tile([128, N], f16, tag="S", name="S")
            lhsT = QAUG[:, bass.ts(t, 128)]
            for kc in range(4):
                P = ppool.tile([128, 2048], f32, tag="P", name="P")
                for i in range(4):
                    nc.tensor.matmul(
                        P[:, bass.ts(i, 512)],
                        lhsT,
                        KAUG[:, 2048 * kc + 512 * i:2048 * kc + 512 * (i + 1)],
                        start=True, stop=True)
                nc.scalar.copy(S[:, bass.ts(kc, 2048)], P[:])
            V8 = VAL8[:, 8 * t:8 * t + 8]
            nc.vector.max(V8, S[:])
            I8 = tp.tile([128, 8], u32, tag="I8", name="I8")
            nc.vector.max_index(I8[:], V8, S[:])
            nc.gpsimd.tensor_copy(GIDX8[:, 8 * t:8 * t + 8], I8[:])

        # ship only ranks 2 and 7 of each tile's top-8 (tie-flag inputs)
        V8R = VAL8.rearrange("p (t k) -> p t k", k=8)
        nc.sync.dma_start(vallo[:, 0:NT], V8R[:, :, 2])
        nc.sync.dma_start(vallo[:, NT:2 * NT], V8R[:, :, 7])
        nc.sync.dma_start(gidxo[:, :], GIDX8[:])

    nc.compile()
    _CACHE["nc"] = nc
    return nc


def _get_runner():
    """Build the 8-core shard_map executable once; return (run, dbg_name)."""
    if "runner" in _CACHE:
        return _CACHE["runner"]

    import jax
    import jax.core
    from jax.experimental.shard_map import shard_map
    from jax.sharding import Mesh, PartitionSpec
    from concourse import bass2jax, mybir

    nc = _build()
    bass2jax.install_neuronx_cc_hook()

    dbg_name = None
    if getattr(nc, "dbg_addr", None) is not None:
        if nc.dbg_callbacks:
            raise RuntimeError("dbg_callbacks unsupported under axon")
        dbg_name = nc.dbg_addr.name
    partition_name = (nc.partition_id_tensor.name
                      if nc.partition_id_tensor else None)

    in_names, out_names, out_avals = [], [], []
    for alloc in nc.m.functions[0].allocations:
        if not isinstance(alloc, mybir.MemoryLocationSet):
            continue
        name = alloc.memorylocations[0].name
        if alloc.kind == "ExternalInput":
            if name != partition_name:
                in_names.append(name)
        elif alloc.kind == "ExternalOutput":
            out_names.append(name)
            out_avals.append(jax.core.ShapedArray(
                tuple(alloc.tensor_shape), mybir.dt.np(alloc.dtype)))
    # the kernel writes every element of every output, so no pre-zeroed
    # donated output operands are needed — results are plain custom-call
    # outputs allocated by the runtime
    bind_in_names = list(in_names)
    if partition_name is not None:
        bind_in_names.append(partition_name)

    def _body(*args):
        operands = list(args)
        if partition_name is not None:
            operands.append(bass2jax.partition_id_tensor())
        outs = bass2jax._bass_exec_p.bind(
            *operands,
            out_avals=tuple(out_avals),
            in_names=tuple(bind_in_names),
            out_names=tuple(out_names),
            lowering_input_output_aliases=(),
            sim_require_finite=True,
            sim_require_nnan=True,
            nc=nc,
        )
        return tuple(outs)

    devices = jax.devices()[:8]
    mesh = Mesh(np.asarray(devices), ("core",))
    in_specs = (PartitionSpec("core"),) * len(in_names)
    out_specs = (PartitionSpec("core"),) * len(out_names)
    sharded = jax.jit(
        shard_map(_body, mesh=mesh, in_specs=in_specs,
                  out_specs=out_specs, check_rep=False),
        keep_unused=True,
    )

    def run(concat_inputs):
        outs = sharded(*[concat_inputs[n] for n in in_names])
        # start all D2H copies before the first blocking asarray so the
        # fetches pipeline into a single axon roundtrip
        for o in outs:
            o.copy_to_host_async()
        return {name: np.asarray(o) for name, o in zip(out_names, outs)}

    _CACHE["parts"] = (sharded, list(in_names), list(out_names), mesh)
    _CACHE["runner"] = (run, dbg_name)
    return _CACHE["runner"]


def _combine_all(pos1, pos2, flow1, val_all, gidx_all):
    """Exact re-rank of device top-8 candidates + weighted warp, all cores.

    val_all: [8*128, 64] f16 (cols 0:32 = rank-2 value per tile, 32:64 =
    rank-7); gidx_all: [8*128, 256] i16.  Core c = 2b+h covers
    pos2[b,:,h*NQ:(h+1)*NQ]; device query (t,p) -> row p, cols 8t..8t+7.
    Returns the full [B, C, N] output.
    """
    vv = np.asarray(val_all).reshape(8, 128, 2, NT)
    idx16 = np.asarray(gidx_all).reshape(8, 128, NT, 8)
    idx = idx16.astype(np.int32)

    # queries q[core, p, t, c] = pos2[b, c, h*4096 + t*128 + p]
    q = np.ascontiguousarray(
        pos2.reshape(B, C, 2, NT, 128).transpose(0, 2, 4, 3, 1)
    ).reshape(8, 128, NT, C)

    # 16-byte rows [kx, ky, kz, |k|^2] viewed as complex128 for fast
    # single-element gathers
    kpad = np.empty((B, N, 4), np.float32)
    kpad[:, :, :3] = (pos1 + flow1).transpose(0, 2, 1)
    kpad[:, :, 3] = np.einsum('bnd,bnd->bn', kpad[..., :3], kpad[..., :3])
    kc128 = kpad.reshape(B * N, 4).view(np.complex128).reshape(B * N)
    fpad = np.zeros((B, N, 4), np.float32)
    fpad[:, :, :3] = flow1.transpose(0, 2, 1)
    fc128 = fpad.reshape(B * N, 4).view(np.complex128).reshape(B * N)

    boff = (np.arange(8, dtype=np.int32) // 2 * N)[:, None, None, None]
    gidx = idx + boff                                        # [8,128,NT,8]
    gk = kc128[gidx].view(np.float32).reshape(8, 128, NT, 8, 4)
    # d2 in the reference's dot form: |q|^2 - 2 q.k + |k|^2
    dot = np.einsum('cptkd,cptd->cptk', gk[..., :3], q,
                    dtype=np.float32, casting='unsafe')
    q2s = np.einsum('cptd,cptd->cpt', q, q)
    d2c = gk[..., 3] - 2.0 * dot + q2s[..., None]            # [8,128,NT,8]

    # exact hierarchical sort key: (f32 d2 bits << 13) | idx.  d2 >= 0 so
    # its bit pattern is order-preserving as an unsigned int.
    key = (d2c.view(np.int32).astype(np.int64) << 13) | idx.astype(np.int64)
    key3 = np.sort(key, axis=-1)[..., :3]
    i3 = (key3 & (N - 1)).astype(np.int32)
    d2_3 = (key3 >> 13).astype(np.int32).view(np.float32)

    # fp16 v3 == v8  =>  candidate set may be incomplete; dup indices too.
    # dups only arise from tied fp16 values, which are adjacent in the
    # sorted top-8, so an adjacency check is exact.
    flag = vv[:, :, 0, :] == vv[:, :, 1, :]
    flag |= (idx16[..., 1:] == idx16[..., :-1]).any(-1)
    if flag.any():
        fc, fp, ft = np.nonzero(flag)
        qf = q[fc, fp, ft]                                   # [m, 3]
        base = (fc // 2) * N
        kall = kpad[..., :3].reshape(B * N, 3)
        for j in range(len(fc)):
            kb = kall[base[j]:base[j] + N]
            d2f = ((qf[j][None, :] - kb) ** 2).sum(-1, dtype=np.float32)
            kf = (d2f.view(np.int32).astype(np.int64) << 13) \
                | np.arange(N, dtype=np.int64)
            k3 = np.sort(kf)[:3]
            i3[fc[j], fp[j], ft[j]] = (k3 & (N - 1)).astype(np.int32)
            d2_3[fc[j], fp[j], ft[j]] = \
                (k3 >> 13).astype(np.int32).view(np.float32)

    dist = np.maximum(np.sqrt(np.maximum(d2_3, 0.0)), EPS)
    inv = 1.0 / dist
    w = inv / inv.sum(-1, keepdims=True)                     # [8,128,NT,3]
    gfl = fc128[i3 + boff].view(np.float32).reshape(
        8, 128, NT, 3, 4)[..., :3]
    flow2 = np.einsum('cptk,cptkd->cptd', w, gfl)            # [8,128,NT,3]
    res = q - flow2
    np.clip(res, -CLAMP, CLAMP, out=res)
    # res[2b+h, p, t, c] -> full[b, c, h*4096 + t*128 + p]
    return np.ascontiguousarray(
        res.reshape(B, 2, 128, NT, C).transpose(0, 4, 1, 3, 2)
    ).reshape(B, C, N)


def kernel(pos1, pos2, flow1):
    pos1 = np.ascontiguousarray(np.asarray(pos1, dtype=np.float32))
    pos2 = np.ascontiguousarray(np.asarray(pos2, dtype=np.float32))
    flow1 = np.ascontiguousarray(np.asarray(flow1, dtype=np.float32))

    run, dbg_name = _get_runner()

    # per-core [3, NQ] query slabs, concatenated on axis 0 (f16 upload —
    # selection only; the exact host re-rank uses the f32 originals)
    q2_all = pos2.reshape(B, C, 2, NQ).transpose(0, 2, 1, 3).astype(
        np.float16).reshape(8 * C, NQ)
    # blocked [32, 768] pre-warped database k = pos1+flow1, repeated for
    # both query halves
    kb_all = np.repeat(
        (pos1 + flow1).reshape(B, C, 32, 256).transpose(0, 2, 1, 3).astype(
            np.float16).reshape(B, 32, 768), 2, axis=0).reshape(8 * 32, 768)

    concat_inputs = {"q2": q2_all, "kb": kb_all}
    if dbg_name is not None:
        concat_inputs[dbg_name] = np.zeros((8, 2), np.uint32)

    outs = run(concat_inputs)
    return _combine_all(pos1, pos2, flow1, outs["vallo"], outs["gidxo"])
